# revision 34
# baseline (speedup 1.0000x reference)
"""Self-contained TRN2 Bass kernel for nn_Attention_26044681683510.

Multi-head attention (B=2, N=2048, C=1024, H=16, rotary, softmax, out-proj),
sharded over 8 NeuronCores as (batch b, head-group hg of 4 heads):
data-parallel on batch, tensor-parallel on heads (column-parallel QKV,
row-parallel out-proj with host-side partial-sum reduction).

All transport is fp16 (11-bit mantissa ~ fp32r precision for this data),
PSUM accumulation fp32.  Cross-engine handoff latency dominates on hardware,
so the attention pipeline keeps 3 independent units in flight (3-deep score
PSUM rotation).  Per-core dataflow:
  A: Q^T/K^T projections into even/odd-channel PSUM tiles (borrowed from the
     score pool rotation); rotary as DVE muls (PSUM fp32 x fp16 cos/sin ->
     fp16 tmp) plus 32-partition Pool/DVE combines that write the per-head
     [A32|B32] layout of qf/kf2 directly — no SBUF rearrange DMAs, no zero
     padding (scores contract K=64).  V projection [seq, vch] with a fused
     ones-column (softmax denominators) runs up front in its own 2-bank pool.
  B: per (head, q-half): scores = kf2-slice.T @ qf-slice -> PSUM[128,1024]
     (3 rotating buffers); exp on ACT (scale 1/8) -> fp16 SBUF; PV
     accumulated over 16 k-tiles into PSUM[65,1024] (row 64 = denominators).
     Normalization: DVE evacuates PV, DVE reciprocal (input must sit at
     partition 0 for the ucode), Pool partition-broadcast, DVE multiply ->
     fp16 attn.  The last head's normalize is split into column halves so
     the out-proj tail starts earlier.
  C: out-proj tail after releasing the attention PSUM pools (6 rotating
     [128,512] accumulators): partials = attn^T.T @ wpT -> fp16 [2048,1024],
     evacuated alternating ACT/DVE (GPSIMD cannot touch PSUM), output DMAs
     on the SP DGE queue.  Host sums the 4 head-group partials per batch in
     fp32 and adds the bias.
"""
import sys

for _p in ("/opt/trn_rl_repo",):
    if _p not in sys.path:
        sys.path.insert(0, _p)

import numpy as np
import concourse.bass as bass
import concourse.mybir as mybir
import concourse.tile as tile
from concourse import bacc

F32 = mybir.dt.float32
F16 = mybir.dt.float16
AFT = mybir.ActivationFunctionType

B, N, C, H = 2, 2048, 1024, 16
D = C // H
HPG = 4               # heads per core
CL = HPG * D          # 256 local channels
NK = C // 128         # 8 contraction tiles for the QKV projections
NSEQ = N // 128       # 16 seq tiles
VW = 66               # per-head v block: [v(64) | ones(1) | pad(1)]
NCORES = 8


def _emit_body(tc, nc, t, uid=""):
    with tc.tile_pool(name=f"cst{uid}", bufs=1) as cst:
        # ---- persistent SBUF tensors ----
        wq_t = [cst.tile([128, 256], F16, name=f"wq{k}{uid}") for k in range(NK)]
        wk_t = [cst.tile([128, 256], F16, name=f"wk{k}{uid}") for k in range(NK)]
        wv_t = [cst.tile([128, 256], F16, name=f"wv{k}{uid}") for k in range(NK)]
        wp_t = [cst.tile([128, 1024], F16, name=f"wp{k}{uid}") for k in range(2)]
        xt = [cst.tile([128, 2048], F16, name=f"x{k}{uid}") for k in range(NK)]
        cos_t = cst.tile([128, 2048], F16, name=f"cos{uid}")
        sin_t = cst.tile([128, 2048], F16, name=f"sin{uid}")
        qf = [cst.tile([128, 2048], F16, name=f"qf{i}{uid}") for i in range(2)]
        kf2 = [cst.tile([128, 2048], F16, name=f"kf{i}{uid}") for i in range(2)]
        v_sb = [cst.tile([128, HPG * VW], F16, name=f"vsb{m}{uid}")
                for m in range(NSEQ)]
        attn_f = [cst.tile([128, 2048], F16, name=f"af{i}{uid}") for i in range(2)]

        # input DMAs, split across the two hardware DGE queues (SP + ACT),
        # ordered so QK blocks 0-1 (seq cols 0-1023) can start earliest
        dq = [nc.sync, nc.scalar]
        for k in range(NK):
            dq[k % 2].dma_start(wk_t[k][:], t["wkT"][128*k:128*(k+1), :])
            dq[(k+1) % 2].dma_start(wq_t[k][:], t["wqT"][128*k:128*(k+1), :])
        nc.sync.dma_start(cos_t[:], t["cosr"][:])
        nc.scalar.dma_start(sin_t[:], t["sinr"][:])
        for quarter in range(4):
            qo = 512 * quarter
            for k in range(NK):
                dq[k % 2].dma_start(xt[k][:, qo:qo+512],
                                    t["xT"][128*k:128*(k+1), qo:qo+512])
            if quarter == 1:
                for k in range(NK):
                    dq[k % 2].dma_start(wv_t[k][:], t["wvT"][128*k:128*(k+1), :])
        for k in range(2):
            dq[k % 2].dma_start(wp_t[k][:], t["wpT"][128*k:128*(k+1), :])
        # ones columns of v_sb (memset whole tile, V copies overwrite cols 0:64)
        for m in range(NSEQ):
            nc.gpsimd.memset(v_sb[m][:], 1.0)

        # ---------------- fused phases: QKV + attention + out-proj ----------
        # Cross-engine handoff latency dominates on hardware, so the score
        # PSUM rotation is 3 deep (6 banks) to keep 3 independent
        # sc->exp->PV units in flight.  PSUM budget (8 banks): scp
        # 3x[128,1024] (6) + pvp 1x[65,1024] (2).  QK projection blocks AND
        # V-projection tiles borrow scp rotation slots, so no separate
        # pools are needed; the attention pools release before the
        # out-proj pool (6x[128,512]) opens for the tail.
        rtp = tc.alloc_tile_pool(name=f"rot{uid}", bufs=3)
        exp_p = tc.alloc_tile_pool(name=f"ex{uid}", bufs=4)
        nrp = tc.alloc_tile_pool(name=f"nr{uid}", bufs=3)
        obp = tc.alloc_tile_pool(name=f"ob{uid}", bufs=6)
        scp = tc.alloc_tile_pool(name=f"scp{uid}", bufs=3, space="PSUM")
        psV = tc.alloc_tile_pool(name=f"psV{uid}", bufs=2, space="PSUM")
        pvp = None
        pop = None

        def emit_qk_block(blk):
            # QK projections for seq block blk (512 cols), one [128,1024]
            # PSUM tile per pre: [pe(512) | po(512)], rows 32h+j = head h,
            # even(2j)/odd(2j+1) channel.  Rotary writes the per-head
            # [A32|B32] layout of qf/kf2 directly.
            cof = 512 * blk
            cs = cos_t[:, cof:cof+512]
            sn = sin_t[:, cof:cof+512]
            for pre, wt in (("k", wk_t), ("q", wq_t)):
                ps = scp.tile([128, 1024], F32, name=f"ps{pre}{blk}{uid}", tag="sc")
                for k in range(NK):
                    nc.tensor.matmul(ps[:, 0:512], wt[k][:, 0:128],
                                     xt[k][:, cof:cof+512],
                                     start=(k == 0), stop=(k == NK - 1))
                for k in range(NK):
                    nc.tensor.matmul(ps[:, 512:1024], wt[k][:, 128:256],
                                     xt[k][:, cof:cof+512],
                                     start=(k == 0), stop=(k == NK - 1))
                pe, po = ps[:, 0:512], ps[:, 512:1024]
                ta = rtp.tile([128, 512], F16, name=f"ta{pre}{blk}{uid}", tag="ta")
                tb = rtp.tile([128, 512], F16, name=f"tb{pre}{blk}{uid}", tag="tb")
                tc2 = rtp.tile([128, 512], F16, name=f"tc{pre}{blk}{uid}", tag="tc")
                td = rtp.tile([128, 512], F16, name=f"td{pre}{blk}{uid}", tag="td")
                nc.vector.tensor_mul(ta[:], pe, cs)
                nc.vector.tensor_mul(tb[:], po, sn)
                nc.vector.tensor_mul(tc2[:], pe, sn)
                nc.vector.tensor_mul(td[:], po, cs)
                dst = qf if pre == "q" else kf2
                for h in range(HPG):
                    r = 32 * h
                    dt_ = dst[h // 2]
                    base = 64 * (h % 2)
                    eng = nc.vector if h == 0 else nc.gpsimd
                    eng.tensor_sub(dt_[base:base+32, cof:cof+512],
                                   ta[r:r+32, :], tb[r:r+32, :])
                    eng.tensor_add(dt_[base+32:base+64, cof:cof+512],
                                   tc2[r:r+32, :], td[r:r+32, :])

        def emit_v(m):
            psv = psV.tile([128, 256], F32, name=f"psv{m}{uid}", tag="psv")
            for k in range(NK):
                nc.tensor.matmul(psv[:], xt[k][:, 128*m:128*(m+1)], wv_t[k][:],
                                 start=(k == 0), stop=(k == NK - 1))
            dst = v_sb[m][:].rearrange("p (h w) -> p h w", h=HPG)[:, :, 0:D]
            src = psv[:].rearrange("p (h d) -> p h d", h=HPG)
            nc.vector.tensor_copy(dst, src)

        def emit_proj(mt):
            po = pop.tile([128, 512], F32, name=f"po{mt}{uid}", tag="po")
            nchunk = mt % 2
            qt = mt // 2
            for k in range(2):
                nc.tensor.matmul(po[:], attn_f[k][:, 128*qt:128*(qt+1)],
                                 wp_t[k][:, 512*nchunk:512*(nchunk+1)],
                                 start=(k == 0), stop=(k == 1))
            ob = obp.tile([128, 512], F16, name=f"ob{mt}{uid}", tag="ob")
            # GPSIMD/Pool cannot access PSUM: alternate ACT/DVE evacuation
            if mt % 2 == 0:
                nc.scalar.copy(ob[:], po[:])
            else:
                nc.vector.tensor_copy(ob[:], po[:])
            nc.sync.dma_start(
                t["outp"][128*qt:128*(qt+1), 512*nchunk:512*(nchunk+1)], ob[:])

        emit_qk_block(0)
        emit_qk_block(1)
        emit_qk_block(2)
        emit_qk_block(3)
        for m in range(NSEQ):
            emit_v(m)
        psV.release()
        pvp = tc.alloc_tile_pool(name=f"pvp{uid}", bufs=1, space="PSUM")

        # Flat software-pipelined attention stream: the scores+exp of unit
        # u+LEAD are emitted before the PV of unit u, so the ACT exp stream
        # stays saturated across (head, q-half) boundaries and the PV-pool
        # evacuation latency hides behind LEAD units of queued PE work.
        # LEAD must stay < scp bufs.
        LEAD = 2
        units = [(qh, hi, h, kt)
                 for qh in range(2) for hi, h in enumerate((0, 2, 1, 3))
                 for kt in range(NSEQ)]
        pvs_by_group = {}
        ex_by_unit = {}

        def emit_sc_exp(u):
            qh, hi, h, kt = units[u]
            ti, off = h // 2, 64 * (h % 2)
            qof = 1024 * qh
            sc = scp.tile([128, 1024], F32, name=f"sc{h}{qh}{kt}{uid}", tag="sc")
            for cc in range(2):
                nc.tensor.matmul(
                    sc[:, 512*cc:512*(cc+1)],
                    kf2[ti][off:off+64, 128*kt:128*(kt+1)],
                    qf[ti][off:off+64, qof+512*cc:qof+512*(cc+1)],
                    start=True, stop=True)
            ex = exp_p.tile([128, 1024], F16, name=f"ex{h}{qh}{kt}{uid}", tag="ex")
            nc.scalar.activation(ex[:], sc[:], AFT.Exp, scale=0.125)
            ex_by_unit[u] = ex

        def emit_pv(u):
            qh, hi, h, kt = units[u]
            ti, off = h // 2, 64 * (h % 2)
            qof = 1024 * qh
            if kt == 0:
                pvs_by_group[(qh, h)] = pvp.tile(
                    [65, 1024], F32, name=f"pv{h}_{qh}{uid}", tag="pv")
            pv = pvs_by_group[(qh, h)]
            ex = ex_by_unit.pop(u)
            for cc in range(2):
                nc.tensor.matmul(pv[:, 512*cc:512*(cc+1)],
                                 v_sb[kt][:, VW*h:VW*h+65],
                                 ex[:, 512*cc:512*(cc+1)],
                                 start=(kt == 0), stop=(kt == NSEQ - 1))
            if kt != NSEQ - 1:
                return
            # evacuate PV promptly (frees the single pv PSUM buffer), then
            # normalize from SBUF off the critical path.  GPSIMD/Pool cannot
            # access PSUM, so copies run on DVE; the reciprocal ucode
            # requires its input at partition 0.  For the final head the
            # chain to the attn mul gates the proj tail, so pull the
            # denominator row out first and split into column halves there.
            if qh == 1 and hi == HPG - 1:
                for cc in range(2):
                    co = 512 * cc
                    rs = nrp.tile([1, 512], F32, name=f"rsL{cc}{uid}", tag="rsL")
                    nc.vector.tensor_copy(rs[:], pv[64:65, co:co+512])
                    rsr = nrp.tile([1, 512], F32, name=f"rsrL{cc}{uid}", tag="rsrL")
                    nc.vector.reciprocal_approx_fast(rsr[:], rs[:])
                    bc = nrp.tile([64, 512], F32, name=f"bcL{cc}{uid}", tag="bcL")
                    nc.gpsimd.partition_broadcast(bc[:], rsr[:], channels=64)
                    nc.vector.tensor_mul(
                        attn_f[ti][off:off+64, qof+co:qof+co+512],
                        pv[0:64, co:co+512], bc[:])
            else:
                pvs = nrp.tile([65, 1024], F32, name=f"pvs{h}{qh}{uid}",
                               tag="pvs")
                nc.vector.tensor_copy(pvs[:], pv[:])
                rs = nrp.tile([1, 1024], F32, name=f"rs{h}{qh}{uid}", tag="rs")
                nc.vector.tensor_copy(rs[:], pvs[64:65, :])
                rsr = nrp.tile([1, 1024], F32, name=f"rsr{h}{qh}{uid}", tag="rsr")
                nc.vector.reciprocal_approx_fast(rsr[:], rs[:])
                bc = nrp.tile([64, 1024], F32, name=f"bc{h}{qh}{uid}", tag="bc")
                nc.gpsimd.partition_broadcast(bc[:], rsr[:], channels=64)
                nc.vector.tensor_mul(attn_f[ti][off:off+64, qof:qof+1024],
                                     pvs[0:64, :], bc[:])

        for u in range(len(units) + LEAD):
            if u < len(units):
                emit_sc_exp(u)
            if u >= LEAD:
                emit_pv(u - LEAD)
        # out-proj tail: release the attention PSUM pools first so the proj
        # pipeline gets 6 banks and runs PE-limited
        pvp.release()
        scp.release()
        pop = tc.alloc_tile_pool(name=f"pop{uid}", bufs=6, space="PSUM")
        for mt in range(32):
            emit_proj(mt)

        if "d_qf0" in t:
            nc.sync.dma_start(t["d_qf0"][:], qf[0][:])
            nc.sync.dma_start(t["d_kf0"][:], kf2[0][:])
            nc.sync.dma_start(t["d_vsb0"][:], v_sb[0][:])
            nc.sync.dma_start(t["d_af0"][:], attn_f[0][:])
            nc.sync.dma_start(t["d_af1"][:], attn_f[1][:])

        for p in (pop, obp, nrp, exp_p, rtp):
            p.release()


def _build_nc(rep=1, num_devices=NCORES, debug_dump=False):
    nc = bacc.Bacc("TRN2", target_bir_lowering=False, debug=False,
                   num_devices=num_devices)
    t = {}
    if debug_dump:
        for nm, shape in (("d_qf0", [128, N]), ("d_kf0", [128, N]),
                          ("d_vsb0", [128, HPG*VW]), ("d_af0", [128, N]),
                          ("d_af1", [128, N])):
            t[nm] = nc.dram_tensor(nm, shape, F16, kind="ExternalOutput").ap()
    t["xT"] = nc.dram_tensor("xT", [C, N], F16, kind="ExternalInput").ap()
    t["wqT"] = nc.dram_tensor("wqT", [C, 256], F16, kind="ExternalInput").ap()
    t["wkT"] = nc.dram_tensor("wkT", [C, 256], F16, kind="ExternalInput").ap()
    t["wvT"] = nc.dram_tensor("wvT", [C, 256], F16, kind="ExternalInput").ap()
    t["wpT"] = nc.dram_tensor("wpT", [256, C], F16, kind="ExternalInput").ap()
    t["cosr"] = nc.dram_tensor("cosr", [128, N], F16, kind="ExternalInput").ap()
    t["sinr"] = nc.dram_tensor("sinr", [128, N], F16, kind="ExternalInput").ap()
    t["outp"] = nc.dram_tensor("outp", [N, C], F16, kind="ExternalOutput").ap()
    with tile.TileContext(nc) as tc:
        for r in range(rep):
            _emit_body(tc, nc, t, uid=f"r{r}" if rep > 1 else "")
    nc.compile()
    return nc


def _make_core_inputs(x, wq, wk, wv, wp, cos, sin, b, hg):
    r0 = CL * hg
    evens = np.concatenate([r0 + D*h + np.arange(0, D, 2) for h in range(HPG)])
    odds = np.concatenate([r0 + D*h + np.arange(1, D, 2) for h in range(HPG)])
    f16 = np.float16
    return {
        "xT": np.ascontiguousarray(x[b].T).astype(f16),
        "wqT": np.ascontiguousarray(wq[np.concatenate([evens, odds])].T).astype(f16),
        "wkT": np.ascontiguousarray(wk[np.concatenate([evens, odds])].T).astype(f16),
        "wvT": np.ascontiguousarray(wv[r0:r0+CL].T).astype(f16),
        "wpT": np.ascontiguousarray(wp[:, r0:r0+CL].T).astype(f16),
        "cosr": np.ascontiguousarray(np.tile(cos.T, (HPG, 1))).astype(f16),
        "sinr": np.ascontiguousarray(np.tile(sin.T, (HPG, 1))).astype(f16),
    }


_CACHE = {}


class _Compiled:
    """Compile once; reusable jitted 8-core SPMD executable (axon/PJRT path)."""

    def __init__(self, nc, n_cores=NCORES):
        import jax
        from jax.sharding import Mesh, PartitionSpec
        from jax.experimental.shard_map import shard_map
        from concourse.bass2jax import (install_neuronx_cc_hook, _bass_exec_p,
                                        partition_id_tensor)
        install_neuronx_cc_hook()
        self.jax = jax
        self.nc = nc
        self.n_cores = n_cores
        in_names, out_names, out_avals, zero_outs = [], [], [], []
        for alloc in nc.m.functions[0].allocations:
            if not isinstance(alloc, mybir.MemoryLocationSet):
                continue
            name = alloc.memorylocations[0].name
            if alloc.kind == "ExternalInput":
                if nc.partition_id_tensor is None or name != nc.partition_id_tensor.name:
                    in_names.append(name)
            elif alloc.kind == "ExternalOutput":
                shape = tuple(alloc.tensor_shape)
                dtype = mybir.dt.np(alloc.dtype)
                out_names.append(name)
                out_avals.append(jax.core.ShapedArray(shape, dtype))
                zero_outs.append(np.zeros(shape, dtype))
        self.in_names, self.out_names = in_names, out_names
        self.out_avals, self.zero_outs = out_avals, zero_outs
        n_params = len(in_names)
        all_in_names = list(in_names) + list(out_names)
        partition_name = nc.partition_id_tensor.name if nc.partition_id_tensor else None
        if partition_name is not None:
            all_in_names.append(partition_name)

        def _body(*args):
            operands = list(args)
            if partition_name is not None:
                operands.append(partition_id_tensor())
            outs = _bass_exec_p.bind(
                *operands, out_avals=tuple(out_avals), in_names=tuple(all_in_names),
                out_names=tuple(out_names), lowering_input_output_aliases=(),
                sim_require_finite=True, sim_require_nnan=True, nc=nc)
            return tuple(outs)

        self.n_params = n_params
        devices = jax.devices()[:n_cores]
        mesh = Mesh(np.asarray(devices), ("core",))
        in_specs = (PartitionSpec("core"),) * (n_params + len(out_names))
        out_specs = (PartitionSpec("core"),) * len(out_names)
        self.fn = jax.jit(
            shard_map(_body, mesh=mesh, in_specs=in_specs, out_specs=out_specs,
                      check_rep=False), keep_unused=True)

    def run(self, in_maps):
        nco = self.n_cores
        concat_in = [np.concatenate([np.asarray(in_maps[c][n]) for c in range(nco)],
                                    axis=0) for n in self.in_names]
        concat_zeros = [np.zeros((nco * z.shape[0], *z.shape[1:]), z.dtype)
                        for z in self.zero_outs]
        outs = self.jax.block_until_ready(self.fn(*concat_in, *concat_zeros))
        return [
            {n: np.asarray(outs[i]).reshape(nco, *self.out_avals[i].shape)[c]
             for i, n in enumerate(self.out_names)}
            for c in range(nco)
        ]


def _get_compiled():
    if "k" not in _CACHE:
        _CACHE["k"] = _Compiled(_build_nc())
    return _CACHE["k"]


def kernel(x, wq, wk, wv, wp, bp, cos, sin, num_heads):
    x = np.asarray(x, dtype=np.float32)
    wq = np.asarray(wq, dtype=np.float32)
    wk = np.asarray(wk, dtype=np.float32)
    wv = np.asarray(wv, dtype=np.float32)
    wp = np.asarray(wp, dtype=np.float32)
    bp = np.asarray(bp, dtype=np.float32)
    cos = np.asarray(cos, dtype=np.float32)
    sin = np.asarray(sin, dtype=np.float32)
    assert int(num_heads) == H, f"kernel hardcodes num_heads={H}"
    assert x.shape == (B, N, C)

    ck = _get_compiled()
    in_maps = [_make_core_inputs(x, wq, wk, wv, wp, cos, sin, c // HPG, c % HPG)
               for c in range(NCORES)]
    results = ck.run(in_maps)
    out = np.zeros((B, N, C), np.float32)
    for c in range(NCORES):
        out[c // HPG] += results[c]["outp"].astype(np.float32)
    out += bp[None, None, :]
    return out
